# revision 24
# baseline (speedup 1.0000x reference)
"""Trainium2 Bass kernel for nn_AdSBHNet (holographic Wilson-loop potential).

Strategy (pure data parallel, 8 NeuronCores, 512 batch rows/core as 4x128):
  * Host (numpy, parameter-only work): polynomial coefficients of f/df/B,
    power series for sqrt(g), z*g'/g, g; scalar bisection prelims
    (zs_max, L_max, L_crit) and a 257-point L->zs init grid. All of that
    depends only on the tiny parameters a, b, logcoef.
  * Quadrature: 64-node Gauss-Legendre on y in [0,1] for the L/dL/V
    connected integrals (the integrands are smooth; the apparent sqrt
    singularities cancel analytically, and h-1 is evaluated through a
    cancellation-free basis).  The disconnected V integral must match the
    reference's trapezoid treatment of the 1/z2^2 endpoint spike, so its
    64 nodes are 31-node GL on the smooth bulk [0, 0.968] plus the
    reference's exact last 33 trapezoid nodes/weights.
  * Device per core: one fp32r matmul per tile per phase (all quadrature
    quantities concatenated into a single PSUM bank: moving operand is
    >=256 wide so fp32r runs at 1 cycle/row), stationaries built from
    exp(k*ln zs) via one IOTA matmul + one wide exp, elementwise work done
    as 4-tile-wide strided ops, and per-tile quadrature sums via grouped
    tensor_reduce (axis=X on a [P,4,64] view).  One Newton step for
    L(zs)=L, then V(zs); T1-T2 is accumulated node-wise as
    sum(SQ*(sqrt(h/m)-1)) to avoid catastrophic cancellation at small zs.
Host gathers the 8 per-core [128,4] outputs and applies the validity mask.
"""
import numpy as np
from math import comb

import concourse.bass as bass
import concourse.tile as tile
from concourse import bacc, mybir
from concourse.bass_utils import run_bass_kernel_spmd
from concourse.hw_specs import get_activation_tables
import bass_rust as _bass_rust


class _PinnedActBacc(bacc.Bacc):
    """Bacc that restricts the activation-table chooser to the single
    natural_log_exp_and_others set (covers Ln/Exp/Square/Copy/Identity)
    so Ln<->Exp transitions never reload tables (~2.7us per reload)."""

    _ACT_SET = "natural_log_exp_and_others"

    def insert_act_table_loads(self):
        has_activation = any(
            isinstance(i, mybir.InstActivation)
            for b in self.main_func.blocks
            for i in b.instructions
        )
        if not has_activation:
            return
        tables = []
        for name, funcs in get_activation_tables(self.m.arch).items():
            tables.append((name, funcs if name == self._ACT_SET else set()))
        _bass_rust.insert_act_table_loads(self, tables)


F32 = np.float32
F64 = np.float64
PI = float(np.pi)
EPS = 1e-12
B_TOTAL = 4096
N_CORES = 8
B_CORE = B_TOTAL // N_CORES      # 512
NT = 4                           # row tiles per core
P = 128                          # partitions
M = 1000                         # reference quadrature points (host only)
N = 64                           # V-phase quadrature nodes per integral
NN = 32                          # Newton-phase quadrature nodes
KS = 8                           # stationary rows: zs^0..4, sA, z4lnz, -zdfs
DT = mybir.dt.float32
DTR = mybir.dt.float32r

_CACHE = {}


# ----------------------------------------------------------------------------
# Host-side math (parameter-only, O(M) work) -- identical to the reference
# ----------------------------------------------------------------------------

def _ygrid():
    return np.linspace(0.001, 0.999, M, dtype=F32).astype(F64)


def _trapz_weights():
    y = _ygrid()
    y0 = y[0]
    h = (y[-1] - y[0]) / (M - 1)
    w = np.full(M, h, F64)
    w[0] = 0.5 * h + y0 + 0.5 * y0 * y0 / h
    w[1] = h - 0.5 * y0 * y0 / h
    w[-1] = 0.5 * h + 0.5 * (1.0 - y[-1])
    return w


def _y2grid():
    return np.linspace(0.001, 1.0, M, dtype=F32).astype(F64)


def _trapz2_weights():
    y2 = _y2grid()
    h2 = (y2[-1] - y2[0]) / (M - 1)
    w2 = np.full(M, h2, F64)
    w2[0] = 0.5 * h2 + 0.5 * y2[0]
    w2[-1] = 0.5 * h2
    return w2, 0.5 * y2[0]


def _f_coeffs(a):
    _a = np.concatenate([np.ones(1, F64), np.asarray(a, F64)])
    A = np.zeros(5, F64)
    q = 0.0
    for i in range(3):
        for j in range(3):
            cc = _a[i] * _a[j]
            if i + j == 4:
                q += -4.0 * cc
            else:
                A[4] += 4.0 * cc / (i + j - 4)
                A[i + j] -= 4.0 * cc / (i + j - 4)
    return A, q


def _df_coeffs(a):
    _a = np.concatenate([np.ones(1, F64), np.asarray(a, F64)])
    A, q = _f_coeffs(a)
    D = 4.0 * A.copy()
    for i in range(3):
        for j in range(3):
            D[i + j] -= 4.0 * _a[i] * _a[j]
    return D, 4.0 * q


def _b_coeffs(a, b):
    last = float(np.asarray(a, F64).sum()) - float(np.asarray(b, F64).sum())
    return np.array([1.0, float(b[0]), float(b[1]), last], F64)


def _series_inv_poly(c, K):
    e = np.zeros(K)
    e[0] = 1.0 / c[0]
    for k in range(1, K):
        s = 0.0
        for j in range(1, min(len(c), k + 1)):
            s += c[j] * e[k - j]
        e[k] = -s / c[0]
    return e


def _conv_trunc(a, b, K):
    return np.convolve(a, b)[:K]


def _build_series(c):
    n_terms = KZ // 4 + 1
    s = np.zeros(n_terms)
    s[0] = 1.0
    for n in range(1, n_terms):
        s[n] = s[n - 1] * (2 * n - 1) / (2 * n)
    rsq = np.zeros(KZ)
    rsq[::4] = s[: len(rsq[::4])]
    sg = _conv_trunc(c, rsq, KZ)                       # B(z)(1-z^4)^-1/2
    zBp = np.array([0.0, c[1], 2 * c[2], 3 * c[3]])
    g1 = 2.0 * _conv_trunc(zBp, _series_inv_poly(c, KZ), KZ)
    g2 = np.zeros(KZ)
    g2[4::4] = 4.0
    gg = g1 + g2                                       # z g'/g
    inv1mz4 = np.zeros(KZ)
    inv1mz4[::4] = 1.0
    gser = _conv_trunc(_conv_trunc(c, c, KZ), inv1mz4, KZ)  # B^2/(1-z^4)
    return sg, gg, gser


class _HostModel:
    """float32 replica of the reference for the scalar bisection prelims."""

    def __init__(self, a, b):
        self.A, self.q = _f_coeffs(a)
        self.D, self.dq = _df_coeffs(a)
        self.c = _b_coeffs(a, b)
        self.y = _ygrid().astype(F32)
        self.u = ((1 - self.y) * (1 + self.y)).astype(F32)
        self.w = _trapz_weights().astype(F32)
        self.y2 = _y2grid().astype(F32)
        w2, c2 = _trapz2_weights()
        self.w2 = w2.astype(F32)
        self.c2 = F32(c2)

    def _f(self, z, lnz):
        A, q = self.A, self.q
        return (A[4] * z**4 + A[3] * z**3 + A[2] * z**2 + A[1] * z + A[0]
                + q * z**4 * lnz).astype(F32)

    def _df(self, z, lnz):
        D, dq = self.D, self.dq
        return (D[0] / z + D[1] + D[2] * z + D[3] * z**2 + D[4] * z**3
                + dq * z**3 * lnz).astype(F32)

    def L_dL(self, zs):
        zs = np.asarray(zs, F32).reshape(-1)[:, None]
        u, y, w = self.u[None, :], self.y[None, :], self.w
        z = (zs * u).astype(F32)
        lnz = np.log(z)
        lnzs = np.log(zs)
        fs = self._f(zs, lnzs)
        dfs = self._df(zs, lnzs)
        rfs = (1.0 / fs).astype(F32)
        f = self._f(z, lnz)
        c = self.c
        Bv = (c[0] + c[1] * z + c[2] * z**2 + c[3] * z**3).astype(F32)
        Bp = (c[1] + 2 * c[2] * z + 3 * c[3] * z**2).astype(F32)
        D_ = (1 - z**4).astype(F32)
        sqrtg = (Bv / np.sqrt(D_)).astype(F32)
        h = (f * rfs / u**4).astype(F32)
        m = np.maximum(h - 1, F32(EPS))
        R = (1.0 / np.sqrt(m)).astype(F32)
        TL = ((sqrtg * R * y * w).sum(-1, dtype=F64)).astype(F32)
        L = (4.0 * zs[:, 0] * TL / PI).astype(F32)
        G = (2 * z * Bp / Bv + 4 * z**4 / D_).astype(F32)
        sA = (zs * dfs * rfs + 2).astype(F32)
        J = (zs**4 / z**3 * self._df(z, lnz) * rfs).astype(F32)
        v = (h * (sA + G) - J - 2 - G).astype(F32)
        IdL = (v * 2 * y * sqrtg * R / m).astype(F32)
        dL = ((IdL * w).sum(-1, dtype=F64) / PI).astype(F32)
        return L, dL

    def V(self, zs, coef):
        zs = np.asarray(zs, F32).reshape(-1)[:, None]
        u, y, w = self.u[None, :], self.y[None, :], self.w
        z = (zs * u).astype(F32)
        lnz = np.log(z)
        lnzs = np.log(zs)
        fs = self._f(zs, lnzs)
        f = self._f(z, lnz)
        c = self.c
        Bv = (c[0] + c[1] * z + c[2] * z**2 + c[3] * z**3).astype(F32)
        g = (Bv * Bv / (1 - z**4)).astype(F32)
        fg = np.maximum(f * g, F32(EPS))
        arg = np.maximum(1 - u**4 * fs / f, F32(EPS))
        integ = (np.sqrt(fg) / u**2 * (1 / np.sqrt(arg) - 1) * y).astype(F32)
        Vc = (coef * PI * 4.0 * (integ * w).sum(-1, dtype=F64) / zs[:, 0]).astype(F32)
        y2, w2 = self.y2[None, :], self.w2
        z2 = (1 - (1 - zs) * y2).astype(F32)
        f2 = self._f(z2, np.log(z2))
        B2 = (c[0] + c[1] * z2 + c[2] * z2**2 + c[3] * z2**3).astype(F32)
        g2 = (B2 * B2 / (1 - z2**4)).astype(F32)
        fg2 = np.maximum(f2 * g2, F32(EPS))
        integ2 = (np.sqrt(fg2) / z2**2).astype(F32)
        Vd = (coef * PI * 2.0 * (1 - zs[:, 0])
              * ((integ2 * w2).sum(-1, dtype=F64) + self.c2)).astype(F32)
        return (Vc - Vd).astype(F32)

    def bisect(self, fun, lo, hi, iters=30):
        lo, hi = F32(lo), F32(hi)
        for _ in range(iters):
            mid = F32(0.5) * (lo + hi)
            if fun(mid) < 0:
                hi = mid
            else:
                lo = mid
        return F32(0.5) * (lo + hi)

    def prelims(self, coef):
        zs_max = self.bisect(lambda mm: self.L_dL(mm)[1][0], 0.001, 0.999)
        L_max = self.L_dL(zs_max)[0][0]
        zs_crit = self.bisect(lambda mm: -self.V(mm, coef)[0], 0.001, zs_max)
        L_crit = self.L_dL(zs_crit)[0][0]
        return zs_max, L_max, zs_crit, L_crit


def _gl_nodes(n):
    x, w = np.polynomial.legendre.leggauss(n)
    return 0.5 * (x + 1.0), 0.5 * w


def _host_build(a, b, logcoef):
    """All parameter-derived constants + the concatenated basis matrix.

    Stationary rows (both phases): 0..4 = zs^k, 5 = sA, 6 = zs^4 ln zs,
    7 = -zs*dfs.  All integrand factors reduce to degree-<=4 polynomial
    blocks in zs (log-domain recombination handles sqrt(g) etc. exactly:
    ln g = 2 ln B(z) - ln(1 - z^4)), so no power series is needed.
    """
    A, q = _f_coeffs(a)
    Dc, dq = _df_coeffs(a)
    c = _b_coeffs(a, b)
    coef = float(np.exp(F32(np.asarray(logcoef).reshape(-1)[0]
                            if np.ndim(logcoef) else logcoef)))

    mdl = _HostModel(a, b)
    zs_max, L_max, zs_crit, L_crit = mdl.prelims(coef)
    zcap = float(min(0.9995, float(zs_max) * 0.97))
    zgrid = np.linspace(1e-4, zcap, 257).astype(F32)
    Lgrid = mdl.L_dL(zgrid)[0]

    def blk(rows, n):
        B = np.zeros((KS, n))
        for k, v in rows.items():
            B[k] = v
        return B

    # ---- Newton GL grid (NN nodes) ----
    y, w = _gl_nodes(NN)
    u = 1 - y * y
    lnu = np.log(u)
    ru = 1.0 / u
    yw = y * w
    one = np.ones(NN)
    # m*fs = sum A_k zs^k (u^{k-4}-1) + q zs^4 ln u   (cancellation-free)
    MB = blk({0: A[0] * (ru**4 - 1), 1: A[1] * (ru**3 - 1),
              2: A[2] * (ru**2 - 1), 3: A[3] * (ru - 1), 4: q * lnu}, NN)
    # (J - (sA-2))*fs
    JF = blk({0: Dc[0] * ru**4, 1: Dc[1] * ru**3, 2: Dc[2] * ru**2,
              3: Dc[3] * ru, 4: Dc[4] + dq * lnu, 6: dq * one,
              7: one}, NN)
    BZW = blk({k: c[k] * u**k * yw for k in range(4)}, NN)      # B(z)*y*w
    D4 = blk({0: one, 4: -u**4}, NN)                            # 1-z^4
    ZBP = blk({k: k * c[k] * u**k * yw for k in range(1, 4)}, NN)  # zB'(z)*yw
    SAM4 = blk({0: -4.0 * one, 5: one}, NN)                     # sA - 4
    # two groups, each read by exactly one consumer class (psum-tile
    # readers serialize): NL -> one fused ACT ln; NR -> the DVE stream.
    # BZW/D4 are duplicated into NR for the reciprocal consumers.
    # zero-pad each group to 256 cols so fp32r matmul runs 1 cycle/row.
    padn = np.zeros((KS, 8 * NN - 3 * NN))
    padn2 = np.zeros((KS, 8 * NN - 5 * NN))
    BNL = np.concatenate([MB, BZW, D4, padn], axis=1)           # [KS, 8*NN]
    BNR = np.concatenate([BZW, D4, JF, ZBP, SAM4, padn2], axis=1)

    # ---- V GL grid (N nodes) ----
    y, w = _gl_nodes(N)
    u = 1 - y * y
    lnu = np.log(u)
    ru = 1.0 / u
    yw = y * w
    one = np.ones(N)
    cw = yw / u**2
    FV = blk({0: A[0] * cw**2, 1: A[1] * u * cw**2, 2: A[2] * u**2 * cw**2,
              3: A[3] * u**3 * cw**2, 4: (A[4] + q * lnu) * u**4 * cw**2,
              6: q * u**4 * cw**2}, N)                          # f(z)*cw^2
    MB2 = blk({0: A[0] * (ru**4 - 1), 1: A[1] * (ru**3 - 1),
               2: A[2] * (ru**2 - 1), 3: A[3] * (ru - 1), 4: q * lnu}, N)
    BZ = blk({k: c[k] * u**k for k in range(4)}, N)             # B(z)
    D4V = blk({0: one, 4: -u**4}, N)                            # 1-z^4
    BVA = np.concatenate([FV, BZ, D4V, np.zeros((KS, N))],
                         axis=1)                                # [KS, 4*N]

    # hybrid disconnected grid: GL bulk + exact reference trapz tail
    h2 = 0.999 / (M - 1)
    n_tail = N // 2
    n_gl = N - n_tail - 1
    y2_B = 0.001 + (M - 1 - n_tail) * h2
    yg, wg = _gl_nodes(n_gl)
    y2 = np.concatenate([yg * y2_B,
                         0.001 + np.arange(M - 1 - n_tail, M) * h2])
    w2 = np.concatenate([wg * y2_B, np.full(n_tail + 1, h2)])
    w2[n_gl] = 0.5 * h2
    w2[-1] = 0.5 * h2
    alpha, beta = 1 - y2, y2
    w2s = w2 * w2

    def phi(coefs, extra, mmax):
        rows = {}
        for mdeg in range(mmax):
            r = np.zeros(N)
            for k in range(mdeg, len(coefs)):
                if coefs[k] != 0:
                    r += coefs[k] * comb(k, mdeg) * alpha**(k - mdeg) * beta**mdeg
            rows[mdeg] = r * extra
        return blk(rows, N)

    Z2B = phi([0, 1], one, 2)
    FDW = phi(list(A), w2s, 5)
    B2D = phi(list(c), one, 4)
    D2B = phi([1, 0, 0, 0, -1], one, 5)
    Z4W = phi([0, 0, 0, 0, 1], w2s, 5)
    BVB = np.concatenate([Z2B, B2D, D2B, MB2, FDW, Z4W],
                         axis=1)                                # [KS, 6*N]

    # prepend-at-0 correction: reference used value 1 at y2=0; the GL bulk
    # integrates the true limit F(0) = sqrt(-f'(1) B(1)^2 / 4)
    fp1 = A[1] + 2 * A[2] + 3 * A[3] + 4 * A[4] + q
    F0 = float(np.sqrt(max(-fp1, 0.0) * float(np.sum(c)) ** 2 / 4.0))
    c2 = 0.5 * 0.001 * (1.0 - F0)

    BASIS = np.concatenate([BNL, BNR, BVA, BVB], axis=1).astype(F32)
    # replicate the 8 stationary-contraction rows at partition offsets
    # 0/32/64/96 so each tile's matmul uses base-partition-aligned operands
    BAS4 = np.zeros((P, BASIS.shape[1]), F32)
    for t in range(NT):
        BAS4[32 * t:32 * t + KS] = BASIS

    return dict(
        A=A, q=q, Dc=Dc, dq=dq, c=c, coef=coef, c2=float(c2),
        zs_max=float(zs_max), L_max=float(L_max), L_crit=float(L_crit),
        zcap=zcap, zgrid=zgrid, Lgrid=Lgrid, BASIS=BAS4,
    )


# ----------------------------------------------------------------------------
# Device graph
# ----------------------------------------------------------------------------

def _build_graph(host):
    A, q, Dc, dq = host["A"], host["q"], host["Dc"], host["dq"]
    coef, c2 = host["coef"], host["c2"]
    zcap = host["zcap"]
    f32 = lambda x: float(F32(x))
    alu = mybir.AluOpType
    act = mybir.ActivationFunctionType
    AX = mybir.AxisListType

    nc = _PinnedActBacc("TRN2", target_bir_lowering=False, debug=False,
                        num_devices=N_CORES)

    BW = 16 * NN + 10 * N            # basis width: 512 + 640
    ltinit_ext = nc.declare_dram_parameter("ltinit", [P, 2 * NT], DT,
                                           isOutput=False)
    basis_ext = nc.declare_dram_parameter("basis", [P, BW], DTR,
                                          isOutput=False)
    ident_ext = nc.declare_dram_parameter("ident", [P, P], DT, isOutput=False)
    out_ext = nc.declare_dram_parameter("out", [P, NT], DT, isOutput=True)

    WN = NT * NN                     # 128: Newton packed width
    WV = NT * N                      # 256: V packed width
    OB = 512                         # psum col offset between tiles (1 bank)
    NQ = 32                          # padded stationary rows per tile

    with tile.TileContext(nc) as tc:
        with (
            tc.tile_pool(name="const", bufs=1) as cpool,
            tc.tile_pool(name="small", bufs=3) as smpool,
            tc.tile_pool(name="sc", bufs=1) as scpool,
            tc.tile_pool(name="stat", bufs=2) as stpool,
            tc.tile_pool(name="wide", bufs=1) as wpool,
            tc.tile_pool(name="psum", bufs=2, space="PSUM") as ppool,
        ):
            LTINIT = cpool.tile([P, 2 * NT], DT, tag="c_ltinit")
            nc.sync.dma_start(LTINIT[:], ltinit_ext[:])
            IDENT = cpool.tile([P, P], DT, tag="c_ident")
            nc.sync.dma_start(IDENT[:], ident_ext[:])
            BASIS = cpool.tile([P, BW], DTR, tag="c_basis")
            nc.sync.dma_start(BASIS[:], basis_ext[:])

            LT = LTINIT[:, 0:NT]
            ZS0 = LTINIT[:, NT:2 * NT]

            def bas(t, lo, hi):
                return BASIS[32 * t:32 * t + KS, lo:hi]

            def small(tag):
                return smpool.tile([P, NT], DT, tag=tag, name=tag)

            def grp_view(t, m):
                return t[:].rearrange("p (t m) -> p t m", m=m)

            def pblk(ps, off, width):
                """per-tile block at col `off` within each 512-col bank."""
                v = ps[:].rearrange("p (t m) -> p t m", m=OB)
                return v[:, :, off:off + width]

            # ============ scalar phase at ZS (shared helper) ============
            def scalar_phase(ZS, newton):
                SC3 = scpool.tile([P, NT * NQ], DT, name="sc3",
                                  tag="sc3n" if newton else "sc3v")
                # q: 0=1, 1=zs, 2=zs^2, 3=zs^3, 4=zs^4, 5=sA, 6=zs^4 lnzs,
                #    7=-zs*dfs; 8..31 zero padding
                nc.vector.memset(SC3[:], 0.0)

                def q_slice(qi):
                    return SC3[:].rearrange("p (t q) -> p t q",
                                            q=NQ)[:, :, qi]

                nc.vector.memset(q_slice(0), 1.0)
                nc.vector.tensor_scalar(q_slice(1), ZS, 1.0, None, alu.mult)
                LNZS = small("lnzs")
                nc.scalar.activation(LNZS[:], ZS, act.Ln)
                ZS2 = q_slice(2)
                nc.vector.tensor_mul(ZS2, ZS, ZS)
                ZS3 = q_slice(3)
                nc.vector.tensor_mul(ZS3, ZS2, ZS)
                ZS4 = q_slice(4)
                nc.vector.tensor_mul(ZS4, ZS2, ZS2)
                LZ4 = q_slice(6)
                nc.vector.tensor_mul(LZ4, ZS4, LNZS[:])
                FS = small("fs")
                t1 = small("tmp1")
                nc.vector.tensor_scalar(t1[:], ZS, f32(A[1]), f32(A[0]),
                                        alu.mult, alu.add)
                t2 = small("tmp2")
                nc.vector.scalar_tensor_tensor(t2[:], ZS2, f32(A[2]), t1[:],
                                               alu.mult, alu.add)
                nc.vector.scalar_tensor_tensor(t1[:], ZS3, f32(A[3]), t2[:],
                                               alu.mult, alu.add)
                nc.vector.scalar_tensor_tensor(t2[:], ZS4, f32(A[4]), t1[:],
                                               alu.mult, alu.add)
                nc.vector.scalar_tensor_tensor(FS[:], LZ4, f32(q), t2[:],
                                               alu.mult, alu.add)
                out = dict(SC3=SC3, FS=FS)
                if newton:
                    LZ3 = small("lz3")
                    nc.vector.tensor_mul(LZ3[:], ZS3, LNZS[:])
                    RZS = small("rzs")
                    nc.vector.reciprocal(RZS[:], ZS)
                    DFS = small("dfs")
                    t3 = small("tmp3")
                    nc.vector.tensor_scalar(t3[:], ZS, f32(Dc[2]), f32(Dc[1]),
                                            alu.mult, alu.add)
                    t4 = small("tmp4")
                    nc.vector.scalar_tensor_tensor(t4[:], ZS2, f32(Dc[3]),
                                                   t3[:], alu.mult, alu.add)
                    nc.vector.scalar_tensor_tensor(t3[:], ZS3, f32(Dc[4]),
                                                   t4[:], alu.mult, alu.add)
                    nc.vector.scalar_tensor_tensor(t4[:], RZS[:], f32(Dc[0]),
                                                   t3[:], alu.mult, alu.add)
                    nc.vector.scalar_tensor_tensor(DFS[:], LZ3[:], f32(dq),
                                                   t4[:], alu.mult, alu.add)
                    RFS = small("rfs")
                    nc.vector.reciprocal(RFS[:], FS[:])
                    LNFS = small("lnfs")
                    nc.scalar.activation(LNFS[:], FS[:], act.Ln)
                    SQFS = small("sqfs")
                    nc.scalar.activation(SQFS[:], LNFS[:], act.Exp, scale=0.5)
                    ZSQ = small("zsq")
                    nc.vector.tensor_mul(ZSQ[:], ZS, SQFS[:])
                    SRT = small("srt")
                    nc.vector.tensor_mul(SRT[:], RFS[:], SQFS[:])
                    LTS = small("lts")
                    nc.vector.tensor_scalar(LTS[:], LT, f32(PI / 2), None,
                                            alu.mult)
                    T0 = small("t0")
                    nc.vector.tensor_mul(T0[:], ZS, DFS[:])
                    nc.vector.tensor_scalar(q_slice(7), T0[:], -1.0,
                                            None, alu.mult)          # -zs*dfs
                    TR = small("tr")
                    nc.vector.tensor_mul(TR[:], T0[:], RFS[:])
                    nc.vector.tensor_scalar(q_slice(5), TR[:], 2.0,
                                            None, alu.add)           # sA
                    out["ZSQ"] = ZSQ
                    out["SRT"] = SRT
                    out["LTS"] = LTS
                return out

            # ============ stationary build: ONE transpose ============
            def build_stationary(SC3, tag):
                TPS = ppool.tile([P, NT * OB], DT, tag="ps", name="tps")
                nc.tensor.transpose(TPS[0:P, 0:P], SC3[:, 0:NT * NQ],
                                    IDENT[:])
                S3 = stpool.tile([P, P], DTR, tag=f"s3_{tag}", name="s3")
                nc.vector.tensor_scalar(S3[:], TPS[0:P, 0:P], 1.0, None,
                                        alu.mult)
                return S3

            # ===================== Newton step =====================
            sc = scalar_phase(ZS0, newton=True)
            S3N = build_stationary(sc["SC3"], "n")
            # NPSL: read once by the fused ln; NPSR: read by the DVE stream
            NPSL = ppool.tile([P, NT * OB], DT, tag="ps", name="npsl")
            for t in range(NT):
                nc.tensor.matmul(NPSL[:, t * OB:t * OB + 8 * NN],
                                 S3N[32 * t:32 * t + KS, :],
                                 bas(t, 0, 8 * NN),
                                 tile_position=(32 * t, 0))
            NPS = ppool.tile([P, NT * OB], DT, tag="ps", name="npsr")
            for t in range(NT):
                nc.tensor.matmul(NPS[:, t * OB:t * OB + 8 * NN],
                                 S3N[32 * t:32 * t + KS, :],
                                 bas(t, 8 * NN, 16 * NN),
                                 tile_position=(32 * t, 0))

            # NPSL blocks: M@0, BZW@NN, D4@2NN
            # NPSR blocks: BZW@0, D4@NN, JF@2NN, ZBP@3NN, SAM4@4NN
            LNX = wpool.tile([P, 3 * WN], DT, tag="w_lnx")
            nc.scalar.activation(grp_view(LNX, 3 * NN),
                                 pblk(NPSL, 0, 3 * NN), act.Ln)
            LNM = grp_view(LNX, 3 * NN)[:, :, 0:NN]
            LBZW = grp_view(LNX, 3 * NN)[:, :, NN:2 * NN]
            LD4 = grp_view(LNX, 3 * NN)[:, :, 2 * NN:3 * NN]
            RBZW = wpool.tile([P, WN], DT, tag="w_rbzw")
            nc.vector.reciprocal(RBZW[:], pblk(NPS, 0, NN))
            RD4 = wpool.tile([P, WN], DT, tag="w_rd4")
            nc.vector.reciprocal(RD4[:], pblk(NPS, NN, NN))
            # E2 = ln(B yw) - .5 ln(1-z^4) - .5 ln(MF); E3 = E2 - ln(MF)
            E1 = wpool.tile([P, WN], DT, tag="w_e1")
            nc.vector.scalar_tensor_tensor(E1[:], LD4, -0.5, LBZW,
                                           alu.mult, alu.add)
            E23 = wpool.tile([P, 2 * WN], DT, tag="w_e23")
            nc.vector.scalar_tensor_tensor(E23[:, 0:WN], LNM, -0.5, E1[:],
                                           alu.mult, alu.add)
            nc.vector.scalar_tensor_tensor(E23[:, WN:2 * WN], LNM, -1.0,
                                           E23[:, 0:WN], alu.mult, alu.add)
            # NPROD = [SW | RSW | P1 | P2]; the batched exp writes the
            # first half directly, so one grouped reduce covers everything
            # (the RSW sums land in unused columns)
            NPROD = wpool.tile([P, 4 * WN], DT, tag="w_nprod")
            nc.scalar.activation(NPROD[:, 0:2 * WN], E23[:], act.Exp)
            SW = NPROD[:, 0:WN]
            RSW = NPROD[:, WN:2 * WN]
            # G + sA = 2 zB'/B + 4/(1-z^4) - 4 + sA
            GA = wpool.tile([P, WN], DT, tag="w_ga")
            nc.vector.tensor_mul(GA[:], pblk(NPS, 3 * NN, NN), RBZW[:])
            T1g = wpool.tile([P, WN], DT, tag="w_t1g")
            nc.vector.scalar_tensor_tensor(T1g[:], RD4[:], 4.0,
                                           pblk(NPS, 4 * NN, NN),
                                           alu.mult, alu.add)
            GSA = wpool.tile([P, WN], DT, tag="w_gsa")
            nc.vector.scalar_tensor_tensor(GSA[:], GA[:], 2.0, T1g[:],
                                           alu.mult, alu.add)
            nc.vector.tensor_mul(NPROD[:, 2 * WN:3 * WN], GSA[:], SW)
            nc.vector.tensor_mul(NPROD[:, 3 * WN:4 * WN],
                                 pblk(NPS, 2 * NN, NN), RSW)
            NRED = smpool.tile([P, 4 * NT], DT, tag="nred")
            nc.vector.tensor_reduce(NRED[:], grp_view(NPROD, NN), AX.X,
                                    alu.add)
            TLp = NRED[:, 0:NT]
            TD1p = NRED[:, 2 * NT:3 * NT]
            TD2p = NRED[:, 3 * NT:4 * NT]

            # zs' = clip(zs - (4/pi zs sqfs TL' - L)(pi/2)/(sqfs TD'))
            TDp = small("tdp")
            nc.vector.tensor_sub(TDp[:], TD1p, TD2p)
            RTD = small("rtd")
            nc.vector.reciprocal(RTD[:], TDp[:])
            T1f = small("t1f")
            nc.vector.tensor_mul(T1f[:], sc["ZSQ"][:], TLp)
            LMF = small("lmf")
            nc.vector.scalar_tensor_tensor(LMF[:], T1f[:], 2.0,
                                           sc["LTS"][:], alu.mult,
                                           alu.subtract)
            D1t = small("d1t")
            nc.vector.tensor_mul(D1t[:], LMF[:], RTD[:])
            DEL = small("del")
            nc.vector.tensor_mul(DEL[:], D1t[:], sc["SRT"][:])
            ZSn = small("zsn")
            nc.vector.tensor_sub(ZSn[:], ZS0, DEL[:])
            ZS1 = smpool.tile([P, NT], DT, tag="zs1")
            nc.vector.tensor_scalar(ZS1[:], ZSn[:], 1e-4, zcap,
                                    alu.max, alu.min)

            # ===================== V phase =====================
            scv = scalar_phase(ZS1[:], newton=False)
            FSV = scv["FS"]
            S3V = build_stationary(scv["SC3"], "v")
            VA0 = 16 * NN
            VB0 = 16 * NN + 4 * N
            VPSA = ppool.tile([P, NT * OB], DT, tag="ps", name="vpsa")
            for t in range(NT):
                nc.tensor.matmul(VPSA[:, t * OB:t * OB + 4 * N],
                                 S3V[32 * t:32 * t + KS, :],
                                 bas(t, VA0, VA0 + 4 * N),
                                 tile_position=(32 * t, 0))
            VPSB = ppool.tile([P, NT * OB], DT, tag="ps", name="vpsb")
            for t in range(NT):
                nc.tensor.matmul(VPSB[:, t * OB:t * OB + 6 * N],
                                 S3V[32 * t:32 * t + KS, :],
                                 bas(t, VB0, VB0 + 6 * N),
                                 tile_position=(32 * t, 0))

            # A blocks: FV@0, BZ@N, D4V@2N (ln only)
            # B blocks: Z2@0, B2D@N, D2@2N, M2@3N, FDW@4N, Z4W@5N
            # B reader order: RMF (critical conn chain), fused ln, TLm, FD2
            RMF = wpool.tile([P, WV], DT, tag="w_rmf")
            nc.vector.reciprocal(RMF[:], pblk(VPSB, 3 * N, N))
            LNA = wpool.tile([P, 3 * WV], DT, tag="w_lna")
            nc.scalar.activation(grp_view(LNA, 3 * N),
                                 pblk(VPSA, 0, 3 * N), act.Ln)
            LFV = grp_view(LNA, 3 * N)[:, :, 0:N]
            LBZ = grp_view(LNA, 3 * N)[:, :, N:2 * N]
            LD4V = grp_view(LNA, 3 * N)[:, :, 2 * N:3 * N]
            LNB = wpool.tile([P, 3 * WV], DT, tag="w_lnb")
            nc.scalar.activation(grp_view(LNB, 3 * N),
                                 pblk(VPSB, 0, 3 * N), act.Ln)
            LZ2 = grp_view(LNB, 3 * N)[:, :, 0:N]
            LB2 = grp_view(LNB, 3 * N)[:, :, N:2 * N]
            LD2 = grp_view(LNB, 3 * N)[:, :, 2 * N:3 * N]
            X = wpool.tile([P, WV], DT, tag="w_x")
            for t in range(NT):
                nc.vector.tensor_scalar(X[:, t * N:(t + 1) * N],
                                        RMF[:, t * N:(t + 1) * N],
                                        FSV[:, t:t + 1], None, alu.mult)
            # batched exp: [ LCC = ln FV + 2 ln B - ln(1-z^4) | ln(1+X) ]
            ELN = wpool.tile([P, 2 * WV], DT, tag="w_eln")
            S1c = wpool.tile([P, WV], DT, tag="w_s1c")
            nc.vector.scalar_tensor_tensor(S1c[:], LBZ, 2.0, LFV,
                                           alu.mult, alu.add)
            nc.gpsimd.tensor_sub(ELN[:, 0:WV], S1c[:], LD4V)
            nc.scalar.activation(ELN[:, WV:2 * WV], X[:], act.Ln,
                                 bias=1.0, scale=1.0)
            EXPC = wpool.tile([P, 2 * WV], DT, tag="w_expc")
            nc.scalar.activation(EXPC[:], ELN[:], act.Exp, scale=0.5)
            SQ = EXPC[:, 0:WV]
            SHM = EXPC[:, WV:2 * WV]

            # disconnected: EXPD = exp(.5(ln FD2 + 2 ln B2D - ln D2 - 4 ln z2))
            # W = 2 ln B2D - ln D2 - 4 ln z2 computed early off-chain (Pool)
            C2t = wpool.tile([P, WV], DT, tag="w_c2t")
            nc.vector.scalar_tensor_tensor(C2t[:], LZ2, 4.0, LD2,
                                           alu.mult, alu.add)
            W2b = wpool.tile([P, WV], DT, tag="w_w2b")
            nc.vector.scalar_tensor_tensor(W2b[:], LB2, 2.0, C2t[:],
                                           alu.mult, alu.subtract)
            TLm = wpool.tile([P, WV], DT, tag="w_tlm")
            nc.vector.tensor_mul(TLm[:], pblk(VPSB, 5 * N, N), LZ2)
            FD2 = wpool.tile([P, WV], DT, tag="w_fd2")
            nc.vector.scalar_tensor_tensor(FD2[:], TLm[:], f32(q),
                                           pblk(VPSB, 4 * N, N),
                                           alu.mult, alu.add)
            LFD2 = wpool.tile([P, WV], DT, tag="w_lfd2")
            nc.scalar.activation(LFD2[:], FD2[:], act.Ln)
            VPROD = wpool.tile([P, 2 * WV], DT, tag="w_vprod")
            nc.vector.scalar_tensor_tensor(VPROD[:, 0:WV], SHM, -1.0, SQ,
                                           alu.add, alu.mult)
            LCOD = wpool.tile([P, WV], DT, tag="w_lcod")
            nc.vector.tensor_add(LCOD[:], LFD2[:], W2b[:])
            nc.scalar.activation(VPROD[:, WV:2 * WV], LCOD[:], act.Exp,
                                 scale=0.5)
            VRED = smpool.tile([P, 2 * NT], DT, tag="vred")
            nc.vector.tensor_reduce(VRED[:], grp_view(VPROD, N), AX.X,
                                    alu.add)
            T12 = VRED[:, 0:NT]
            TDd = VRED[:, NT:2 * NT]

            # ---- finalize ----
            RZSV = small("rzsv")
            nc.vector.reciprocal(RZSV[:], ZS1[:])
            VC1 = small("vc1")
            nc.vector.tensor_mul(VC1[:], T12, RZSV[:])
            O1 = small("o1")
            nc.vector.tensor_scalar(O1[:], VC1[:], f32(4.0 * PI * coef),
                                    None, alu.mult)
            TVD = small("tvd")
            nc.vector.tensor_scalar(TVD[:], TDd, f32(c2), None, alu.add)
            OMZ = small("omz")
            nc.vector.tensor_scalar(OMZ[:], ZS1[:], -1.0, 1.0,
                                    alu.mult, alu.add)
            VD1 = small("vd1")
            nc.vector.tensor_mul(VD1[:], TVD[:], OMZ[:])
            OUT = small("outt")
            nc.vector.scalar_tensor_tensor(OUT[:], VD1[:],
                                           f32(-2.0 * PI * coef), O1[:],
                                           alu.mult, alu.add)
            nc.sync.dma_start(out_ext[:], OUT[:])

    nc.compile()
    return nc


# ----------------------------------------------------------------------------
# kernel entry point
# ----------------------------------------------------------------------------

def kernel(Ls, a, b, logcoef):
    Ls_in = np.asarray(Ls, F32).reshape(-1)
    n_in = Ls_in.size
    if n_in == B_TOTAL:
        Ls = Ls_in
    else:
        Ls = np.full(B_TOTAL, 0.05, F32)
        Ls[:min(n_in, B_TOTAL)] = Ls_in[:B_TOTAL]
    a = np.asarray(a, F32).reshape(-1)
    b = np.asarray(b, F32).reshape(-1)

    host = _host_build(a, b, logcoef)

    L_crit = F32(host["L_crit"])
    valid = Ls < L_crit
    L_eff = np.where(valid, Ls, F32(0.5) * L_crit).astype(F32)
    Lg, zg = host["Lgrid"], host["zgrid"]
    if np.all(np.diff(Lg) > 0):
        init = np.interp(L_eff, Lg, zg).astype(F32)
    else:
        init = np.clip(L_eff / F32(host["L_max"]) * F32(host["zs_max"]),
                       1e-4, 0.9995).astype(F32)

    key = ("graph2", host["BASIS"].tobytes(), F32(host["zcap"]).tobytes(),
           F32(host["coef"]).tobytes(), F32(host["c2"]).tobytes())
    kh = hash(key)
    if kh not in _CACHE:
        _CACHE[kh] = _build_graph(host)
    nc = _CACHE[kh]

    consts = dict(
        basis=host["BASIS"],
        ident=np.eye(P, dtype=F32),
    )

    in_maps = []
    for i in range(N_CORES):
        sl = slice(i * B_CORE, (i + 1) * B_CORE)
        lt = np.ascontiguousarray(L_eff[sl].reshape(NT, P).T)
        zi = np.ascontiguousarray(init[sl].reshape(NT, P).T)
        in_maps.append(dict(
            ltinit=np.concatenate([lt, zi], axis=1),
            **consts,
        ))

    res = run_bass_kernel_spmd(nc, in_maps, list(range(N_CORES)))
    globals()["_LAST_RESULTS"] = res

    V = np.empty(B_TOTAL, F32)
    for i in range(N_CORES):
        V[i * B_CORE:(i + 1) * B_CORE] = res.results[i]["out"].T.ravel()

    out = np.where(valid, V, np.zeros_like(V)).astype(F32)
    if n_in != B_TOTAL:
        full = np.zeros(n_in, F32)
        full[:min(n_in, B_TOTAL)] = out[:min(n_in, B_TOTAL)]
        return full
    return out
